# revision 12
# baseline (speedup 1.0000x reference)
"""Causal self-attention (B=2, T=2048, C=1024, H=16) on 8 TRN2 NeuronCores.

Sharding: data-parallel over batch (2 groups of 4 cores) x tensor-parallel
over heads (4 heads per core, Megatron-style column/row split of the
qkv / proj weights). Each core computes, for its (batch, head-group):

    qT      = W_q^T x^T + b_q            [256, 2048]   (transposed layout)
    kT      = W_k^T x^T                  [256, 2048]   (k bias dropped: only
                                          the query bias shifts softmax --
                                          per-query terms cancel)
    v       = x W_v + b_v                [2048, 256]
    S^T     = kT-tiles x qT              per (head, i-chunk) blocks
    P       = exp(S^T / 8) * causal_mask
    Yu^T    = [v | 1]^T P^T              rows 0..63 unnormalized y^T,
                                          row 64 = softmax denominator
    y^T     = Yu^T * (1/denom)           (denom broadcast across partitions
                                          via gpsimd partition_broadcast)
    out^T  += W_p-rows^T y^T             [1024, 2048] partial projection

The host sums the 4 partial projections per batch and adds b_proj.

All matmul operands are bfloat16 (fp32 PSUM accumulation): bf16 enables
fast-weight-load so back-to-back matmuls run at ~1 col/cycle instead of
being LDWEIGHTS-bound. The two K=64 S matmuls of a head pair are issued
into disjoint PE row groups (partitions 0-63 / 64-127) so they execute
concurrently. exp runs once per head-pair j-tile on a [128, 1024] PSUM
pair tile. Projection / next-chunk qkv matmuls are interleaved between
attention tiles as filler so the PE never waits on the ACT engine.
"""

import numpy as np

B, T, C, H = 2, 2048, 1024, 16
HD = C // H  # 64
HG = 4  # head-groups (tensor-parallel degree); B * HG = 8 cores
HPG = H // HG  # heads per group = 4
DG = HPG * HD  # columns per group = 256
TC = 512  # t-chunk (moving free dim)
NTC = T // TC  # 4
NJT = T // 128  # 16 j-tiles of 128 keys

_cached = {}
_DEBUG = False


def _build():
    import concourse.mybir as mybir
    import concourse.tile as tile
    from concourse import bacc

    F32 = mybir.dt.float32
    BF16 = mybir.dt.bfloat16
    Act = mybir.ActivationFunctionType

    nc = bacc.Bacc()
    x_d = nc.declare_dram_parameter("xt", [C, T], BF16, isOutput=False)
    wqk_d = nc.declare_dram_parameter("wqk", [C, 2 * DG], BF16, isOutput=False)
    bq_d = nc.declare_dram_parameter("bq", [DG, 1], F32, isOutput=False)
    wv_d = nc.declare_dram_parameter("wv", [C, DG], BF16, isOutput=False)
    bv_d = nc.declare_dram_parameter("bv", [1, DG], F32, isOutput=False)
    wp_d = nc.declare_dram_parameter("wp", [DG, C], BF16, isOutput=False)
    out_d = nc.declare_dram_parameter("outt", [C, T], BF16, isOutput=True)
    if _DEBUG:
        qkdbg_d = nc.declare_dram_parameter("qkdbg", [128, 4 * T], BF16, isOutput=True)
        v1dbg_d = nc.declare_dram_parameter(
            "v1dbg", [128, NJT * HPG * (HD + 1)], BF16, isOutput=True
        )
        ytdbg_d = nc.declare_dram_parameter("ytdbg", [128, 2 * T], BF16, isOutput=True)

    with tile.TileContext(nc) as tc:
        with (
            tc.tile_pool(name="const", bufs=1) as const,
            tc.tile_pool(name="sb", bufs=1) as sb,
            tc.tile_pool(name="ps", bufs=1, space="PSUM") as ps,
        ):
            # ---- constants ----
            # single [128,128] diagonal-block mask: keep where i' >= j
            mask_f = const.tile([128, 128], F32)
            nc.vector.memset(mask_f, 1.0)
            nc.gpsimd.affine_select(
                out=mask_f,
                in_=mask_f,
                compare_op=mybir.AluOpType.is_ge,
                fill=0.0,
                base=0,
                pattern=[[1, 128]],
                channel_multiplier=-1,
            )
            mask = const.tile([128, 128], BF16)
            nc.vector.tensor_copy(mask, mask_f)

            ones_f = const.tile([128, 1], F32)
            nc.vector.memset(ones_f, 1.0)
            for _ in range(12):
                pw = ps.tile([128, TC], F32, tag="mm512", bufs=2)
                nc.tensor.matmul(
                    pw[:, 0:128], mask_f, mask_f, start=True, stop=True
                )

            # x arrives pre-transposed from the host: [C, T], c on partitions
            def build_xt(tcx):
                xt = sb.tile([128, 8, TC], BF16, tag="xt", bufs=2, name=f"xt{tcx}")
                for kc in range(8):
                    nc.sync.dma_start(
                        out=xt[:, kc, :],
                        in_=x_d[
                            kc * 128 : (kc + 1) * 128,
                            tcx * TC : (tcx + 1) * TC,
                        ],
                    )
                return xt

            xts = [build_xt(0)]

            # ---- weights ----
            wqk_sb = const.tile([128, 8, 2 * DG], BF16)
            wv_sb = const.tile([128, 8, DG], BF16)
            wp_sb = const.tile([128, 2, C], BF16)
            bq_sb = const.tile([128, 2], F32)
            bv_sb = const.tile([128, DG], F32)
            for kc in range(8):
                nc.scalar.dma_start(
                    out=wqk_sb[:, kc, :], in_=wqk_d[kc * 128 : (kc + 1) * 128, :]
                )
                nc.scalar.dma_start(
                    out=wv_sb[:, kc, :], in_=wv_d[kc * 128 : (kc + 1) * 128, :]
                )
            for cc in range(2):
                nc.scalar.dma_start(
                    out=wp_sb[:, cc, :], in_=wp_d[cc * 128 : (cc + 1) * 128, :]
                )
            for jt in range(2):
                nc.scalar.dma_start(
                    out=bq_sb[:, jt : jt + 1],
                    in_=bq_d[jt * 128 : (jt + 1) * 128, :],
                )
            nc.gpsimd.dma_start(out=bv_sb, in_=bv_d[:].to_broadcast((128, DG)))

            # ---- persistent activations ----
            # qkT [j, t]: j-tiles 0,1 = q (heads 0..3), 2,3 = k
            qk_sb = const.tile([128, 4, T], BF16)
            # v1 [t, d]: per (t-tile, head): 64 v columns + ones column
            v1_sb = const.tile([128, NJT, HPG, HD + 1], BF16)
            nc.vector.memset(v1_sb[:, :, :, HD : HD + 1], 1.0)
            # y^T [c', t]: c' = head-major 256 rows in 2 tiles
            yt_sb = const.tile([128, 2, T], BF16)

            # ---------- phase-1 / projection units (also used as filler) ----
            def qkT_unit(c, jt):
                # one output j-tile of q or k for chunk c
                xt = xts[c]
                pqk = ps.tile([128, TC], F32, tag="mm512", bufs=2)
                for kc in range(8):
                    nc.tensor.matmul(
                        pqk,
                        wqk_sb[:, kc, jt * 128 : (jt + 1) * 128],
                        xt[:, kc, :],
                        start=(kc == 0),
                        stop=(kc == 7),
                    )
                dst = qk_sb[:, jt, c * TC : (c + 1) * TC]
                if jt < 2:  # q tiles get the query bias
                    nc.vector.tensor_scalar_add(dst, pqk, bq_sb[:, jt : jt + 1])
                else:  # k tiles raw
                    nc.vector.tensor_copy(dst, pqk)

            def v_unit(c, tt):
                xt = xts[c]
                pv = ps.tile([128, TC], F32, tag="mm512", bufs=2)
                for kc in range(8):
                    nc.tensor.matmul(
                        pv[:, 0:DG],
                        xt[:, kc, tt * 128 : (tt + 1) * 128],
                        wv_sb[:, kc, :],
                        start=(kc == 0),
                        stop=(kc == 7),
                    )
                for h in range(HPG):
                    nc.vector.tensor_add(
                        v1_sb[:, c * 4 + tt, h, 0:HD],
                        pv[:, h * HD : (h + 1) * HD],
                        bv_sb[:, h * HD : (h + 1) * HD],
                    )

            def proj_unit(c, mt):
                po = ps.tile([128, TC], F32, tag="mm512", bufs=2)
                for cc in range(2):
                    nc.tensor.matmul(
                        po,
                        wp_sb[:, cc, mt * 128 : (mt + 1) * 128],
                        yt_sb[:, cc, c * TC : (c + 1) * TC],
                        start=(cc == 0),
                        stop=(cc == 1),
                    )
                ot = sb.tile([128, TC], BF16, tag="ot", bufs=6)
                nc.vector.tensor_copy(ot, po)
                eng = nc.sync if mt % 2 == 0 else nc.scalar
                eng.dma_start(
                    out=out_d[mt * 128 : (mt + 1) * 128, c * TC : (c + 1) * TC],
                    in_=ot,
                )

            # ---------------- main loop over i-chunks ----------------------
            for tcx in range(NTC):
                if tcx == 0:
                    for jt in range(4):
                        qkT_unit(0, jt)
                    for tt in range(4):
                        v_unit(0, tt)

                # prefetch next chunk's x^T and queue its phase-1 + the
                # previous chunk's projection as filler between attention
                # tiles (keeps the PE fed while ACT runs exp)
                fillers = []
                if tcx + 1 < NTC:
                    xts.append(build_xt(tcx + 1))
                    p1 = [lambda jt=jt: qkT_unit(tcx + 1, jt) for jt in range(4)]
                    p1 += [lambda tt=tt: v_unit(tcx + 1, tt) for tt in range(4)]
                else:
                    p1 = []
                if tcx == 1:
                    pr = [lambda mt=mt: proj_unit(0, mt) for mt in range(8)]
                elif tcx == NTC - 1:
                    pr = [
                        lambda c=c, mt=mt: proj_unit(c, mt)
                        for c in (1, 2)
                        for mt in range(8)
                    ]
                else:
                    pr = []
                for i in range(max(len(p1), len(pr))):
                    if i < len(p1):
                        fillers.append(p1[i])
                    if i < len(pr):
                        fillers.append(pr[i])

                q = tcx
                njt = 4 * (q + 1)
                total_iters = 2 * njt
                it = 0
                nfill = 0
                qcols = slice(q * TC, (q + 1) * TC)

                for p in range(2):  # head pairs (2p, 2p+1)
                    py = [
                        ps.tile([HD + 1, TC], F32, tag="py", bufs=2, name=f"py{p}{h}")
                        for h in range(2)
                    ]
                    pending = []  # (jt, cut, p_sb) awaiting their PV matmuls
                    for jt in range(njt):
                        # pace filler emission across the chunk
                        want = (it * len(fillers)) // total_iters if fillers else 0
                        while nfill < want:
                            fillers[nfill]()
                            nfill += 1
                        it += 1

                        k = jt - 4 * q
                        cut = 0 if k <= 0 else 128 * k
                        spair = ps.tile([128, 2 * TC], F32, tag="spair", bufs=2)
                        for h in range(2):
                            nc.tensor.matmul(
                                spair[:, h * TC + cut : (h + 1) * TC],
                                qk_sb[
                                    h * 64 : h * 64 + 64,
                                    2 + p,
                                    jt * 128 : (jt + 1) * 128,
                                ],
                                qk_sb[h * 64 : h * 64 + 64, p, q * TC + cut : (q + 1) * TC],
                                start=True,
                                stop=True,
                                tile_position=(h * 64, 0),
                            )
                        p_sb = sb.tile([128, 2 * TC], BF16, tag="p", bufs=8)
                        nc.scalar.activation(
                            p_sb[:, cut:], spair[:, cut:], Act.Exp, scale=0.125
                        )
                        if k >= 0:  # diagonal block: mask the 128-col window
                            for h in range(2):
                                ms = slice(h * TC + cut, h * TC + cut + 128)
                                nc.vector.tensor_mul(
                                    p_sb[:, ms], p_sb[:, ms], mask
                                )
                        pending.append((jt, cut, p_sb))
                        if len(pending) > 4:
                            pjt, pcut, pp = pending.pop(0)
                            for h in range(2):
                                nc.tensor.matmul(
                                    py[h][:, pcut:],
                                    v1_sb[:, pjt, 2 * p + h, :],
                                    pp[:, h * TC + pcut : (h + 1) * TC],
                                    start=(pjt == 0),
                                    stop=False,
                                )
                    for pjt, pcut, pp in pending:
                        for h in range(2):
                            nc.tensor.matmul(
                                py[h][:, pcut:],
                                v1_sb[:, pjt, 2 * p + h, :],
                                pp[:, h * TC + pcut : (h + 1) * TC],
                                start=(pjt == 0),
                                stop=(pjt == njt - 1),
                            )
                    # normalization: recip of denominator row, broadcast
                    # across partitions on gpsimd, multiply (no PE involved)
                    for h in range(2):
                        dn = sb.tile([1, TC], F32, tag="dn", bufs=4)
                        nc.scalar.activation(dn, py[h][HD : HD + 1, :], Act.Copy)
                        recip = sb.tile([1, TC], F32, tag="recip", bufs=4)
                        nc.vector.reciprocal_approx_fast(recip, dn)
                        bp = sb.tile([HD, TC], F32, tag="bp", bufs=4)
                        nc.gpsimd.partition_broadcast(bp, recip)
                        nc.vector.tensor_mul(
                            yt_sb[h * 64 : h * 64 + 64, p, qcols],
                            py[h][0:HD, :],
                            bp,
                        )

                while nfill < len(fillers):
                    fillers[nfill]()
                    nfill += 1

            for mt in range(8):
                proj_unit(NTC - 1, mt)

            if _DEBUG:
                nc.sync.dma_start(
                    out=qkdbg_d[:, :], in_=qk_sb[:, :, :]
                )
                nc.sync.dma_start(
                    out=v1dbg_d[:, :],
                    in_=v1_sb[:, :, :, :],
                )
                nc.sync.dma_start(
                    out=ytdbg_d[:, :], in_=yt_sb[:, :, :]
                )

    nc.finalize()
    return nc


def _in_maps(x, W_attn, b_attn, W_proj):
    import ml_dtypes

    bf16 = ml_dtypes.bfloat16
    in_maps = []
    for core in range(8):
        b = core // HG
        hg = core % HG
        qs, ks, vs = hg * DG, C + hg * DG, 2 * C + hg * DG
        wqk = np.concatenate(
            [W_attn[:, qs : qs + DG], W_attn[:, ks : ks + DG]], axis=1
        )
        in_maps.append(
            {
                "xt": np.ascontiguousarray(x[b].T).astype(bf16),
                "wqk": np.ascontiguousarray(wqk).astype(bf16),
                "bq": np.ascontiguousarray(
                    b_attn[qs : qs + DG].reshape(DG, 1)
                ).astype(np.float32),
                "wv": np.ascontiguousarray(W_attn[:, vs : vs + DG]).astype(bf16),
                "bv": np.ascontiguousarray(
                    b_attn[vs : vs + DG].reshape(1, DG)
                ).astype(np.float32),
                "wp": np.ascontiguousarray(W_proj[hg * DG : (hg + 1) * DG, :]).astype(
                    bf16
                ),
            }
        )
    return in_maps


def _combine(results, b_proj):
    out = np.empty((B, T, C), dtype=np.float32)
    for b in range(B):
        acc = results[4 * b]["outt"].astype(np.float32)
        for hg in range(1, HG):
            acc = acc + results[4 * b + hg]["outt"].astype(np.float32)
        out[b] = acc.T + b_proj
    return out


def get_nc():
    if "nc" not in _cached:
        _cached["nc"] = _build()
    return _cached["nc"]


def kernel(x, W_attn, b_attn, W_proj, b_proj):
    from concourse.bass_utils import run_bass_kernel_spmd

    nc = get_nc()
    x = np.asarray(x, dtype=np.float32)
    W_attn = np.asarray(W_attn, dtype=np.float32)
    b_attn = np.asarray(b_attn, dtype=np.float32)
    W_proj = np.asarray(W_proj, dtype=np.float32)
    b_proj = np.asarray(b_proj, dtype=np.float32)

    in_maps = _in_maps(x, W_attn, b_attn, W_proj)
    r = run_bass_kernel_spmd(nc, in_maps, core_ids=list(range(8)), trace=False)
    return _combine(r.results, b_proj)


# revision 13
# speedup vs baseline: 1.0058x; 1.0058x over previous
"""Causal self-attention (B=2, T=2048, C=1024, H=16) on 8 TRN2 NeuronCores.

Sharding: data-parallel over batch (2 groups of 4 cores) x tensor-parallel
over heads (4 heads per core, Megatron-style column/row split of the
qkv / proj weights). Each core computes, for its (batch, head-group):

    qT      = W_q^T x^T + b_q            [256, 2048]   (transposed layout)
    kT      = W_k^T x^T                  [256, 2048]   (k bias dropped: only
                                          the query bias shifts softmax --
                                          per-query terms cancel)
    v       = x W_v + b_v                [2048, 256]
    S^T     = kT-tiles x qT              per (head, i-chunk) blocks
    P       = exp(S^T / 8) * causal_mask
    Yu^T    = [v | 1]^T P^T              rows 0..63 unnormalized y^T,
                                          row 64 = softmax denominator
    y^T     = Yu^T * (1/denom)           (denom broadcast across partitions
                                          via gpsimd partition_broadcast)
    out^T  += W_p-rows^T y^T             [1024, 2048] partial projection

The host sums the 4 partial projections per batch and adds b_proj.

All matmul operands are bfloat16 (fp32 PSUM accumulation): bf16 enables
fast-weight-load so back-to-back matmuls run at ~1 col/cycle instead of
being LDWEIGHTS-bound. The two K=64 S matmuls of a head pair are issued
into disjoint PE row groups (partitions 0-63 / 64-127) so they execute
concurrently. exp runs once per head-pair j-tile on a [128, 1024] PSUM
pair tile. Projection / next-chunk qkv matmuls are interleaved between
attention tiles as filler so the PE never waits on the ACT engine.
"""

import numpy as np

B, T, C, H = 2, 2048, 1024, 16
HD = C // H  # 64
HG = 4  # head-groups (tensor-parallel degree); B * HG = 8 cores
HPG = H // HG  # heads per group = 4
DG = HPG * HD  # columns per group = 256
TC = 512  # t-chunk (moving free dim)
NTC = T // TC  # 4
NJT = T // 128  # 16 j-tiles of 128 keys

_cached = {}
_DEBUG = False


def _build():
    import concourse.mybir as mybir
    import concourse.tile as tile
    from concourse import bacc

    F32 = mybir.dt.float32
    BF16 = mybir.dt.bfloat16
    Act = mybir.ActivationFunctionType

    nc = bacc.Bacc()
    x_d = nc.declare_dram_parameter("xt", [C, T], BF16, isOutput=False)
    wqk_d = nc.declare_dram_parameter("wqk", [C, 2 * DG], BF16, isOutput=False)
    bq_d = nc.declare_dram_parameter("bq", [DG, 1], F32, isOutput=False)
    wv_d = nc.declare_dram_parameter("wv", [C, DG], BF16, isOutput=False)
    bv_d = nc.declare_dram_parameter("bv", [1, DG], F32, isOutput=False)
    wp_d = nc.declare_dram_parameter("wp", [DG, C], BF16, isOutput=False)
    out_d = nc.declare_dram_parameter("outt", [C, T], BF16, isOutput=True)
    if _DEBUG:
        qkdbg_d = nc.declare_dram_parameter("qkdbg", [128, 4 * T], BF16, isOutput=True)
        v1dbg_d = nc.declare_dram_parameter(
            "v1dbg", [128, NJT * HPG * (HD + 1)], BF16, isOutput=True
        )
        ytdbg_d = nc.declare_dram_parameter("ytdbg", [128, 2 * T], BF16, isOutput=True)

    with tile.TileContext(nc) as tc:
        with (
            tc.tile_pool(name="const", bufs=1) as const,
            tc.tile_pool(name="sb", bufs=1) as sb,
            tc.tile_pool(name="ps", bufs=1, space="PSUM") as ps,
        ):
            # ---- constants ----
            # single [128,128] diagonal-block mask: keep where i' >= j
            mask_f = const.tile([128, 128], F32)
            nc.vector.memset(mask_f, 1.0)
            nc.gpsimd.affine_select(
                out=mask_f,
                in_=mask_f,
                compare_op=mybir.AluOpType.is_ge,
                fill=0.0,
                base=0,
                pattern=[[1, 128]],
                channel_multiplier=-1,
            )
            mask = const.tile([128, 128], BF16)
            nc.vector.tensor_copy(mask, mask_f)

            ones_f = const.tile([128, 1], F32)
            nc.vector.memset(ones_f, 1.0)
            for _ in range(12):
                pw = ps.tile([128, TC], F32, tag="mm512", bufs=2)
                nc.tensor.matmul(
                    pw[:, 0:128], mask_f, mask_f, start=True, stop=True
                )

            # x arrives pre-transposed from the host: [C, T], c on partitions
            def build_xt(tcx):
                xt = sb.tile([128, 8, TC], BF16, tag="xt", bufs=2, name=f"xt{tcx}")
                for kc in range(8):
                    nc.sync.dma_start(
                        out=xt[:, kc, :],
                        in_=x_d[
                            kc * 128 : (kc + 1) * 128,
                            tcx * TC : (tcx + 1) * TC,
                        ],
                    )
                return xt

            xts = [build_xt(0)]

            # ---- weights ----
            wqk_sb = const.tile([128, 8, 2 * DG], BF16)
            wv_sb = const.tile([128, 8, DG], BF16)
            wp_sb = const.tile([128, 2, C], BF16)
            bq_sb = const.tile([128, 2], F32)
            bv_sb = const.tile([128, DG], F32)
            for kc in range(8):
                nc.scalar.dma_start(
                    out=wqk_sb[:, kc, :], in_=wqk_d[kc * 128 : (kc + 1) * 128, :]
                )
                nc.scalar.dma_start(
                    out=wv_sb[:, kc, :], in_=wv_d[kc * 128 : (kc + 1) * 128, :]
                )
            for cc in range(2):
                nc.scalar.dma_start(
                    out=wp_sb[:, cc, :], in_=wp_d[cc * 128 : (cc + 1) * 128, :]
                )
            for jt in range(2):
                nc.scalar.dma_start(
                    out=bq_sb[:, jt : jt + 1],
                    in_=bq_d[jt * 128 : (jt + 1) * 128, :],
                )
            nc.gpsimd.dma_start(out=bv_sb, in_=bv_d[:].to_broadcast((128, DG)))

            # ---- persistent activations ----
            # qkT [j, t]: j-tiles 0,1 = q (heads 0..3), 2,3 = k
            qk_sb = const.tile([128, 4, T], BF16)
            # v1 [t, d]: per (t-tile, head): 64 v columns + ones column
            v1_sb = const.tile([128, NJT, HPG, HD + 1], BF16)
            nc.vector.memset(v1_sb[:, :, :, HD : HD + 1], 1.0)
            # y^T [c', t]: c' = head-major 256 rows in 2 tiles
            yt_sb = const.tile([128, 2, T], BF16)

            # ---------- phase-1 / projection units (also used as filler) ----
            def qkT_unit(c, jt):
                # one output j-tile of q or k for chunk c
                xt = xts[c]
                pqk = ps.tile([128, TC], F32, tag="mm512", bufs=2)
                for kc in range(8):
                    nc.tensor.matmul(
                        pqk,
                        wqk_sb[:, kc, jt * 128 : (jt + 1) * 128],
                        xt[:, kc, :],
                        start=(kc == 0),
                        stop=(kc == 7),
                    )
                dst = qk_sb[:, jt, c * TC : (c + 1) * TC]
                if jt < 2:  # q tiles get the query bias
                    nc.vector.tensor_scalar_add(dst, pqk, bq_sb[:, jt : jt + 1])
                else:  # k tiles raw
                    nc.vector.tensor_copy(dst, pqk)

            def v_unit(c, tt):
                xt = xts[c]
                pv = ps.tile([128, TC], F32, tag="mm512", bufs=2)
                for kc in range(8):
                    nc.tensor.matmul(
                        pv[:, 0:DG],
                        xt[:, kc, tt * 128 : (tt + 1) * 128],
                        wv_sb[:, kc, :],
                        start=(kc == 0),
                        stop=(kc == 7),
                    )
                for h in range(HPG):
                    nc.vector.tensor_add(
                        v1_sb[:, c * 4 + tt, h, 0:HD],
                        pv[:, h * HD : (h + 1) * HD],
                        bv_sb[:, h * HD : (h + 1) * HD],
                    )

            def proj_unit(c, mt):
                po = ps.tile([128, TC], F32, tag="mm512", bufs=2)
                for cc in range(2):
                    nc.tensor.matmul(
                        po,
                        wp_sb[:, cc, mt * 128 : (mt + 1) * 128],
                        yt_sb[:, cc, c * TC : (c + 1) * TC],
                        start=(cc == 0),
                        stop=(cc == 1),
                    )
                ot = sb.tile([128, TC], BF16, tag="ot", bufs=6)
                nc.vector.tensor_copy(ot, po)
                eng = nc.sync if mt % 2 == 0 else nc.scalar
                eng.dma_start(
                    out=out_d[mt * 128 : (mt + 1) * 128, c * TC : (c + 1) * TC],
                    in_=ot,
                )

            # ---------------- main loop over i-chunks ----------------------
            for tcx in range(NTC):
                if tcx == 0:
                    for jt in range(4):
                        qkT_unit(0, jt)
                    for tt in range(4):
                        v_unit(0, tt)

                # prefetch next chunk's x^T and queue its phase-1 + the
                # previous chunk's projection as filler between attention
                # tiles (keeps the PE fed while ACT runs exp)
                fillers = []
                if tcx + 1 < NTC:
                    xts.append(build_xt(tcx + 1))
                    p1 = [lambda jt=jt: qkT_unit(tcx + 1, jt) for jt in range(4)]
                    if tcx + 1 < NTC - 1:
                        p1 += [lambda tt=tt: v_unit(tcx + 1, tt) for tt in range(4)]
                else:
                    # final chunk: its own deferred v-units lead the filler
                    # list (consumed only by PV of j-tiles 12-15, late in
                    # the chunk), easing the ACT-bound tail
                    p1 = [lambda tt=tt: v_unit(NTC - 1, tt) for tt in range(4)]
                if tcx == 1:
                    pr = [lambda mt=mt: proj_unit(0, mt) for mt in range(8)]
                elif tcx == NTC - 1:
                    pr = [
                        lambda c=c, mt=mt: proj_unit(c, mt)
                        for c in (1, 2)
                        for mt in range(8)
                    ]
                else:
                    pr = []
                for i in range(max(len(p1), len(pr))):
                    if i < len(p1):
                        fillers.append(p1[i])
                    if i < len(pr):
                        fillers.append(pr[i])

                q = tcx
                njt = 4 * (q + 1)
                total_iters = 2 * njt
                it = 0
                nfill = 0
                qcols = slice(q * TC, (q + 1) * TC)

                for p in range(2):  # head pairs (2p, 2p+1)
                    py = [
                        ps.tile([HD + 1, TC], F32, tag="py", bufs=2, name=f"py{p}{h}")
                        for h in range(2)
                    ]
                    pending = []  # (jt, cut, p_sb) awaiting their PV matmuls
                    for jt in range(njt):
                        # pace filler emission across the chunk
                        want = (it * len(fillers)) // total_iters if fillers else 0
                        while nfill < want:
                            fillers[nfill]()
                            nfill += 1
                        it += 1

                        k = jt - 4 * q
                        cut = 0 if k <= 0 else 128 * k
                        spair = ps.tile([128, 2 * TC], F32, tag="spair", bufs=2)
                        for h in range(2):
                            nc.tensor.matmul(
                                spair[:, h * TC + cut : (h + 1) * TC],
                                qk_sb[
                                    h * 64 : h * 64 + 64,
                                    2 + p,
                                    jt * 128 : (jt + 1) * 128,
                                ],
                                qk_sb[h * 64 : h * 64 + 64, p, q * TC + cut : (q + 1) * TC],
                                start=True,
                                stop=True,
                                tile_position=(h * 64, 0),
                            )
                        p_sb = sb.tile([128, 2 * TC], BF16, tag="p", bufs=8)
                        nc.scalar.activation(
                            p_sb[:, cut:], spair[:, cut:], Act.Exp, scale=0.125
                        )
                        if k >= 0:  # diagonal block: mask the 128-col window
                            for h in range(2):
                                ms = slice(h * TC + cut, h * TC + cut + 128)
                                nc.vector.tensor_mul(
                                    p_sb[:, ms], p_sb[:, ms], mask
                                )
                        pending.append((jt, cut, p_sb))
                        if len(pending) > 4:
                            pjt, pcut, pp = pending.pop(0)
                            for h in range(2):
                                nc.tensor.matmul(
                                    py[h][:, pcut:],
                                    v1_sb[:, pjt, 2 * p + h, :],
                                    pp[:, h * TC + pcut : (h + 1) * TC],
                                    start=(pjt == 0),
                                    stop=False,
                                )
                    for pjt, pcut, pp in pending:
                        for h in range(2):
                            nc.tensor.matmul(
                                py[h][:, pcut:],
                                v1_sb[:, pjt, 2 * p + h, :],
                                pp[:, h * TC + pcut : (h + 1) * TC],
                                start=(pjt == 0),
                                stop=(pjt == njt - 1),
                            )
                    # normalization: recip of denominator row, broadcast
                    # across partitions on gpsimd, multiply (no PE involved)
                    for h in range(2):
                        dn = sb.tile([1, TC], F32, tag="dn", bufs=4)
                        nc.scalar.activation(dn, py[h][HD : HD + 1, :], Act.Copy)
                        recip = sb.tile([1, TC], F32, tag="recip", bufs=4)
                        nc.vector.reciprocal_approx_fast(recip, dn)
                        bp = sb.tile([HD, TC], F32, tag="bp", bufs=4)
                        nc.gpsimd.partition_broadcast(bp, recip)
                        nc.vector.tensor_mul(
                            yt_sb[h * 64 : h * 64 + 64, p, qcols],
                            py[h][0:HD, :],
                            bp,
                        )

                while nfill < len(fillers):
                    fillers[nfill]()
                    nfill += 1

            for mt in range(8):
                proj_unit(NTC - 1, mt)

            if _DEBUG:
                nc.sync.dma_start(
                    out=qkdbg_d[:, :], in_=qk_sb[:, :, :]
                )
                nc.sync.dma_start(
                    out=v1dbg_d[:, :],
                    in_=v1_sb[:, :, :, :],
                )
                nc.sync.dma_start(
                    out=ytdbg_d[:, :], in_=yt_sb[:, :, :]
                )

    nc.finalize()
    return nc


def _in_maps(x, W_attn, b_attn, W_proj):
    import ml_dtypes

    bf16 = ml_dtypes.bfloat16
    in_maps = []
    for core in range(8):
        b = core // HG
        hg = core % HG
        qs, ks, vs = hg * DG, C + hg * DG, 2 * C + hg * DG
        wqk = np.concatenate(
            [W_attn[:, qs : qs + DG], W_attn[:, ks : ks + DG]], axis=1
        )
        in_maps.append(
            {
                "xt": np.ascontiguousarray(x[b].T).astype(bf16),
                "wqk": np.ascontiguousarray(wqk).astype(bf16),
                "bq": np.ascontiguousarray(
                    b_attn[qs : qs + DG].reshape(DG, 1)
                ).astype(np.float32),
                "wv": np.ascontiguousarray(W_attn[:, vs : vs + DG]).astype(bf16),
                "bv": np.ascontiguousarray(
                    b_attn[vs : vs + DG].reshape(1, DG)
                ).astype(np.float32),
                "wp": np.ascontiguousarray(W_proj[hg * DG : (hg + 1) * DG, :]).astype(
                    bf16
                ),
            }
        )
    return in_maps


def _combine(results, b_proj):
    out = np.empty((B, T, C), dtype=np.float32)
    for b in range(B):
        acc = results[4 * b]["outt"].astype(np.float32)
        for hg in range(1, HG):
            acc = acc + results[4 * b + hg]["outt"].astype(np.float32)
        out[b] = acc.T + b_proj
    return out


def get_nc():
    if "nc" not in _cached:
        _cached["nc"] = _build()
    return _cached["nc"]


def kernel(x, W_attn, b_attn, W_proj, b_proj):
    from concourse.bass_utils import run_bass_kernel_spmd

    nc = get_nc()
    x = np.asarray(x, dtype=np.float32)
    W_attn = np.asarray(W_attn, dtype=np.float32)
    b_attn = np.asarray(b_attn, dtype=np.float32)
    W_proj = np.asarray(W_proj, dtype=np.float32)
    b_proj = np.asarray(b_proj, dtype=np.float32)

    in_maps = _in_maps(x, W_attn, b_attn, W_proj)
    r = run_bass_kernel_spmd(nc, in_maps, core_ids=list(range(8)), trace=False)
    return _combine(r.results, b_proj)
